# revision 31
# baseline (speedup 1.0000x reference)
"""DynamicLinear (MoE routing) Trainium2 Bass kernel.

Math (per sample b):
    out[b] = sum_k attn[b,k] * (x[b] @ W[k].T + bias[k])

Sharding: 8 cores in a 2x4 grid over (batch, out_features).
Each core computes out[b_half, o_quarter] from x[b_half] and
W[:, o_quarter, :] -- no cross-core communication.

Precision-hybrid contraction: per expert k, the first _N8K[k] (of 16)
128-wide contraction blocks run as fp8e4m3 DoubleRow matmuls (2
blocks per instruction, double-pumped PE = 2x bf16 rate), the rest
in bf16.  The total fp8 scale _SX*_SW = 2^17 is a power of two, so
the bf16 operands are pre-scaled by 2^17 exactly (lossless) and both
precisions accumulate into ONE PSUM bank per (expert, b_tile); the
inverse scale rides in the per-expert attn combine scalars
(attn_s = attn / 2^17).  _SW lifts the uniform weights out of
e4m3's subnormal range.  Full-batch rel_l2 vs fp32 ref: ~1.75e-2
(gate 2e-2); HW matches the CPU model to 7 digits.

Schedule: expert-outer sweeps.  k0 is split: its fp8 half runs first
(prologue needs only ~300KB before the first matmul -> early PE
start), its bf16 half runs last (epilogue).  k1..k3 run as merged
sweeps (DR pairs + bf16 blocks in one accumulation group).  One DVE
combine per group plus one DVE bias op per (k,t), paced well under
the PE; PSUM banks round-robin depth 8.
"""

import numpy as np

_B, _K, _IN, _OUT = 4096, 4, 2048, 2048
_GRID_B, _GRID_O = 2, 4
_BL = _B // _GRID_B      # 2048 batch rows per core
_OL = _OUT // _GRID_O    # 512 out cols per core
_NBT = _BL // 128        # 16 b tiles
_NIT = _IN // 128        # 16 contraction tiles
_N8K = (6, 6, 6, 4)      # fp8 blocks per expert (each even)
_N8MAX = max(_N8K)
_X16LO = min(_N8K)       # x16 tile holds blocks _X16LO.._NIT-1
_NX16 = _NIT - _X16LO
_STOT = 131072.0         # 2^17: total fp8 scale; bf16 w pre-scaled by it
_SX = 24.0
_SW = _STOT / _SX

_CACHE = {}
LAST_RESULTS = None


def _build_program():
    import concourse.bass as bass
    import concourse.tile as tile
    from concourse import bacc, mybir

    f32 = mybir.dt.float32
    bf16 = mybir.dt.bfloat16
    fp8 = mybir.dt.float8e4
    MULT = mybir.AluOpType.mult
    ADD = mybir.AluOpType.add
    DR = mybir.MatmulPerfMode.DoubleRow

    nc = bacc.Bacc("TRN2", target_bir_lowering=False, debug=False)
    xT8 = nc.dram_tensor("xT8", [_NBT, 128, _N8MAX, 128], fp8,
                         kind="ExternalInput").ap()
    xT16 = nc.dram_tensor("xT16", [_NBT, 128, _NX16, 128], bf16,
                          kind="ExternalInput").ap()
    attn = nc.dram_tensor("attn", [_BL, _K], f32, kind="ExternalInput").ap()
    attn_s = nc.dram_tensor("attn_s", [_BL, _K], f32,
                            kind="ExternalInput").ap()
    # per-expert fp8 W (ragged across k -> one tensor per k)
    wT8 = [nc.dram_tensor(f"wT8_{k}", [128, _N8K[k], _OL], fp8,
                          kind="ExternalInput").ap() for k in range(_K)]
    wT16 = [nc.dram_tensor(f"wT16_{k}", [128, _NIT - _N8K[k], _OL], bf16,
                           kind="ExternalInput").ap() for k in range(_K)]
    bias = nc.dram_tensor("bias", [_K, _OL], f32, kind="ExternalInput").ap()
    out = nc.dram_tensor("out", [_BL, _OL], f32, kind="ExternalOutput").ap()

    with tile.TileContext(nc) as tc:
        with (
            tc.tile_pool(name="w8", bufs=_K) as w8p,
            tc.tile_pool(name="w16",
                         bufs=sum(-(-(_NIT - n) // 4) for n in _N8K)) as w16p,
            tc.tile_pool(name="x8", bufs=_NBT) as x8p,
            tc.tile_pool(name="x16", bufs=_NBT) as x16p,
            tc.tile_pool(name="singles", bufs=1) as singles,
            tc.tile_pool(name="acc", bufs=_NBT) as accp,
            tc.tile_pool(name="psum", bufs=8, space="PSUM") as psump,
        ):
            def load_w8(k, chunks=1):
                # chunked so the very first matmul waits on minimal bytes
                t_ = w8p.tile([128, _N8K[k], _OL], fp8, tag="w8",
                              name=f"w8_{k}")
                step = _N8K[k] // chunks
                for lo in range(0, _N8K[k], step):
                    nc.sync.dma_start(out=t_[:, lo:lo + step, :],
                                      in_=wT8[k][:, lo:lo + step, :])
                return t_

            def load_w16(k):
                # granules of <=4 blocks so a sweep can start on granule 0
                # while the rest still streams
                nb = _NIT - _N8K[k]
                gs = []
                for lo in range(0, nb, 4):
                    n = min(4, nb - lo)
                    t_ = w16p.tile([128, n, _OL], bf16, tag="w16",
                                   name=f"w16_{k}_{lo}")
                    nc.sync.dma_start(out=t_, in_=wT16[k][:, lo:lo + n, :])
                    gs.append(t_)
                return gs

            def load_attn(src, nm):
                t_ = singles.tile([128, _NBT, _K], f32, tag=nm, name=nm)
                nc.sync.dma_start(
                    out=t_,
                    in_=bass.AP(
                        tensor=src.tensor,
                        offset=src.offset,
                        ap=[[_K, 128], [128 * _K, _NBT], [1, _K]],
                    ),
                )
                return t_

            # ---- loads in need-order ----
            w8t = {0: load_w8(0, chunks=3)}
            x8ts = {}
            for t in range(_NBT):
                t_ = x8p.tile([128, _N8MAX, 128], fp8, tag="x8",
                              name=f"x8_{t}")
                if t == 0:
                    # first matmul waits only on the first pair's 32KB
                    nc.scalar.dma_start(out=t_[:, 0:2, :],
                                        in_=xT8[t][:, 0:2, :])
                    nc.scalar.dma_start(out=t_[:, 2:, :],
                                        in_=xT8[t][:, 2:, :])
                else:
                    nc.scalar.dma_start(out=t_, in_=xT8[t])
                x8ts[t] = t_
            attn_s_sb = load_attn(attn_s, "attn_s_sb")
            w8t[1] = load_w8(1)
            attn_sb = load_attn(attn, "attn_sb")
            w8t[2] = load_w8(2)
            w8t[3] = load_w8(3)
            x16ts = {}
            for t in range(_NBT):
                t_ = x16p.tile([128, _NX16, 128], bf16, tag="x16",
                               name=f"x16_{t}")
                nc.scalar.dma_start(out=t_, in_=xT16[t])
                x16ts[t] = t_
            # bf16 W in sweep-consumption order: k2, k3 (merged sweeps run
            # right after the fp8 phases), then k1, then k0 (epilogue)
            w16t = {}
            for k in (2, 3, 1, 0):
                w16t[k] = load_w16(k)

            bias_rep = singles.tile([128, _K, _OL], f32, tag="bias_rep",
                                    name="bias_rep")
            nc.gpsimd.dma_start(
                out=bias_rep,
                in_=bass.AP(
                    tensor=bias.tensor,
                    offset=bias.offset,
                    ap=[[0, 128], bias.ap[0], bias.ap[1]],
                ),
            )

            acc = {}

            def dr_passes(ps, k, t, start):
                for j in range(_N8K[k] // 2):
                    nc.tensor.matmul(
                        ps,
                        lhsT=x8ts[t][:, 2 * j:2 * j + 2, :],
                        rhs=w8t[k][:, 2 * j:2 * j + 2, :],
                        start=(start and j == 0), stop=False,
                        perf_mode=DR,
                    )

            def bf16_passes(ps, k, t, start):
                nb = _NIT - _N8K[k]
                xoff = _N8K[k] - _X16LO
                for ii in range(nb):
                    nc.tensor.matmul(
                        ps,
                        lhsT=x16ts[t][:, xoff + ii, :],
                        rhs=w16t[k][ii // 4][:, ii % 4, :],
                        start=(start and ii == 0), stop=(ii == nb - 1),
                    )

            # ---- fp8 prologue: experts 0 and 1 per b_tile (6 DR instrs
            # per 96KB x8 tile halves the cold-DMA demand rate; needs only
            # ~1MiB of weights before the bf16 stream is flowing) ----
            for t in range(_NBT):
                for k in (0, 1):
                    ps = psump.tile([128, _OL], f32, tag="ps",
                                    name=f"psA{k}_{t}")
                    np8 = _N8K[k] // 2
                    for j in range(np8):
                        nc.tensor.matmul(
                            ps,
                            lhsT=x8ts[t][:, 2 * j:2 * j + 2, :],
                            rhs=w8t[k][:, 2 * j:2 * j + 2, :],
                            start=(j == 0), stop=(j == np8 - 1),
                            perf_mode=DR,
                        )
                    as_sc = attn_s_sb[:, t, k:k + 1]
                    if k == 0:
                        at = accp.tile([128, _OL], f32, tag="acc",
                                       name=f"acc{t}")
                        acc[t] = at
                        nc.vector.tensor_scalar(
                            out=at, in0=ps, scalar1=as_sc,
                            scalar2=None, op0=MULT,
                        )
                    else:
                        nc.vector.scalar_tensor_tensor(
                            out=acc[t], in0=ps, scalar=as_sc,
                            in1=acc[t], op0=MULT, op1=ADD,
                        )

            # ---- merged sweeps: experts 2,3: fp8 + bf16 in one group ----
            for k in (2, 3):
                for t in range(_NBT):
                    ps = psump.tile([128, _OL], f32, tag="ps",
                                    name=f"ps_{k}_{t}")
                    dr_passes(ps, k, t, start=True)
                    bf16_passes(ps, k, t, start=False)
                    nc.vector.scalar_tensor_tensor(
                        out=acc[t], in0=ps, scalar=attn_s_sb[:, t, k:k + 1],
                        in1=acc[t], op0=MULT, op1=ADD,
                    )
                    nc.vector.scalar_tensor_tensor(
                        out=acc[t], in0=bias_rep[:, k, :],
                        scalar=attn_sb[:, t, k:k + 1], in1=acc[t],
                        op0=MULT, op1=ADD,
                    )

            # ---- bf16 halves of experts 1 then 0; bias; store at the end --
            for k in (1, 0):
                for t in range(_NBT):
                    ps = psump.tile([128, _OL], f32, tag="ps",
                                    name=f"psE_{k}_{t}")
                    # bias is independent of ps: issue first so the final
                    # tail is just one combine + the store
                    nc.vector.scalar_tensor_tensor(
                        out=acc[t], in0=bias_rep[:, k, :],
                        scalar=attn_sb[:, t, k:k + 1], in1=acc[t],
                        op0=MULT, op1=ADD,
                    )
                    bf16_passes(ps, k, t, start=True)
                    nc.vector.scalar_tensor_tensor(
                        out=acc[t], in0=ps, scalar=attn_s_sb[:, t, k:k + 1],
                        in1=acc[t], op0=MULT, op1=ADD,
                    )
                    if k == 0:
                        nc.sync.dma_start(
                            out=out[t * 128:(t + 1) * 128, :], in_=acc[t],
                        )

    nc.compile()
    return nc


def _get_program():
    if "nc" not in _CACHE:
        _CACHE["nc"] = _build_program()
    return _CACHE["nc"]


def _ensure_axon_hooks_importable():
    """bass_utils' trace branch imports antenv.axon_hooks, which the
    trimmed agent image may lack; stub it (hook=None) so a stray
    BASS_TRACE=1 degrades to an untraced run instead of crashing."""
    import sys
    import types

    try:
        import antenv.axon_hooks  # noqa: F401
        return
    except ImportError:
        pass
    mod = types.ModuleType("antenv.axon_hooks")
    mod._hook = None
    mod.get_axon_ntff_profile_hook = lambda: mod._hook

    def _set(h):
        mod._hook = h

    mod.set_axon_ntff_profile_hook = _set
    sys.modules["antenv.axon_hooks"] = mod
    try:
        import antenv
        antenv.axon_hooks = mod
    except ImportError:
        pass


def kernel(**inputs):
    global LAST_RESULTS
    import ml_dtypes
    from concourse.bass_utils import run_bass_kernel_spmd

    _ensure_axon_hooks_importable()

    x = np.ascontiguousarray(inputs["x"], dtype=np.float32)
    attn = np.ascontiguousarray(inputs["softmax_attention"], dtype=np.float32)
    w = np.ascontiguousarray(inputs["weight"], dtype=np.float32)
    b = np.ascontiguousarray(inputs["bias"], dtype=np.float32)

    nc = _get_program()
    in_maps = []
    for c in range(8):
        gb, go = divmod(c, _GRID_O)
        x_sl = x[gb * _BL:(gb + 1) * _BL]
        w_sl = w[:, go * _OL:(go + 1) * _OL, :]
        # tile-contiguous device layouts (see _build_program):
        # xT*[t, i_in, j, b_in] = x[t*128 + b_in, j*128 + i_in] (* scale)
        # wT*[i_in, j, o]       = W[k, o, j*128 + i_in] (* scale)
        xT8 = np.ascontiguousarray(
            (x_sl[:, :_N8MAX * 128] * _SX).T
            .reshape(_N8MAX, 128, _NBT, 128).transpose(2, 1, 0, 3)
        ).astype(ml_dtypes.float8_e4m3)
        xT16 = np.ascontiguousarray(
            x_sl[:, _X16LO * 128:].T
            .reshape(_NX16, 128, _NBT, 128).transpose(2, 1, 0, 3)
        ).astype(ml_dtypes.bfloat16)
        attn_c = np.ascontiguousarray(attn[gb * _BL:(gb + 1) * _BL])
        im = {
            "xT8": xT8,
            "xT16": xT16,
            "attn": attn_c,
            "attn_s": np.ascontiguousarray(attn_c / _STOT),
            "bias": np.ascontiguousarray(b[:, go * _OL:(go + 1) * _OL]),
        }
        for k in range(_K):
            cut = _N8K[k] * 128
            im[f"wT8_{k}"] = np.ascontiguousarray(
                (w_sl[k, :, :cut] * _SW).T
                .reshape(_N8K[k], 128, _OL).transpose(1, 0, 2)
            ).astype(ml_dtypes.float8_e4m3)
            im[f"wT16_{k}"] = np.ascontiguousarray(
                (w_sl[k, :, cut:] * _STOT).T
                .reshape(_NIT - _N8K[k], 128, _OL).transpose(1, 0, 2)
            ).astype(ml_dtypes.bfloat16)
        in_maps.append(im)

    res = run_bass_kernel_spmd(nc, in_maps, list(range(8)))
    LAST_RESULTS = res

    full = np.empty((_B, _OUT), dtype=np.float32)
    for c in range(8):
        gb, go = divmod(c, _GRID_O)
        full[gb * _BL:(gb + 1) * _BL, go * _OL:(go + 1) * _OL] = \
            res.results[c]["out"]
    return full


# revision 32
# speedup vs baseline: 1.1555x; 1.1555x over previous
"""DynamicLinear (MoE routing) Trainium2 Bass kernel.

Math (per sample b):
    out[b] = sum_k attn[b,k] * (x[b] @ W[k].T + bias[k])

Sharding: 8 cores in a 2x4 grid over (batch, out_features).
Each core computes out[b_half, o_quarter] from x[b_half] and
W[:, o_quarter, :] -- no cross-core communication.

Precision-hybrid contraction: per expert k, the first _N8K[k] (of 16)
128-wide contraction blocks run as fp8e4m3 DoubleRow matmuls (2
blocks per instruction, double-pumped PE = 2x bf16 rate), the rest
in bf16.  The total fp8 scale _SX*_SW = 2^17 is a power of two, so
the bf16 operands are pre-scaled by 2^17 exactly (lossless) and both
precisions accumulate into ONE PSUM bank per (expert, b_tile); the
inverse scale rides in the per-expert attn combine scalars
(attn_s = attn / 2^17).  _SW lifts the uniform weights out of
e4m3's subnormal range.  Full-batch rel_l2 vs fp32 ref: ~1.75e-2
(gate 2e-2); HW matches the CPU model to 7 digits.

Schedule: expert-outer sweeps.  k0 is split: its fp8 half runs first
(prologue needs only ~300KB before the first matmul -> early PE
start), its bf16 half runs last (epilogue).  k1..k3 run as merged
sweeps (DR pairs + bf16 blocks in one accumulation group).  One DVE
combine per group plus one DVE bias op per (k,t), paced well under
the PE; PSUM banks round-robin depth 8.
"""

import numpy as np

_B, _K, _IN, _OUT = 4096, 4, 2048, 2048
_GRID_B, _GRID_O = 2, 4
_BL = _B // _GRID_B      # 2048 batch rows per core
_OL = _OUT // _GRID_O    # 512 out cols per core
_NBT = _BL // 128        # 16 b tiles
_NIT = _IN // 128        # 16 contraction tiles
_N8K = (6, 6, 6, 4)      # fp8 blocks per expert (each even)
_N8MAX = max(_N8K)
_X16LO = min(_N8K)       # x16 tile holds blocks _X16LO.._NIT-1
_NX16 = _NIT - _X16LO
_STOT = 131072.0         # 2^17: total fp8 scale; bf16 w pre-scaled by it
_SX = 24.0
_SW = _STOT / _SX

_CACHE = {}
LAST_RESULTS = None


def _build_program():
    import concourse.bass as bass
    import concourse.tile as tile
    from concourse import bacc, mybir

    f32 = mybir.dt.float32
    bf16 = mybir.dt.bfloat16
    fp8 = mybir.dt.float8e4
    MULT = mybir.AluOpType.mult
    ADD = mybir.AluOpType.add
    DR = mybir.MatmulPerfMode.DoubleRow

    nc = bacc.Bacc("TRN2", target_bir_lowering=False, debug=False)
    xT8 = nc.dram_tensor("xT8", [_NBT, 128, _N8MAX, 128], fp8,
                         kind="ExternalInput").ap()
    xT16 = nc.dram_tensor("xT16", [_NBT, 128, _NX16, 128], bf16,
                          kind="ExternalInput").ap()
    attn = nc.dram_tensor("attn", [_BL, _K], f32, kind="ExternalInput").ap()
    attn_s = nc.dram_tensor("attn_s", [_BL, _K], f32,
                            kind="ExternalInput").ap()
    # per-expert fp8 W (ragged across k -> one tensor per k)
    wT8 = [nc.dram_tensor(f"wT8_{k}", [128, _N8K[k], _OL], fp8,
                          kind="ExternalInput").ap() for k in range(_K)]
    wT16 = [nc.dram_tensor(f"wT16_{k}", [128, _NIT - _N8K[k], _OL], bf16,
                           kind="ExternalInput").ap() for k in range(_K)]
    bias = nc.dram_tensor("bias", [_K, _OL], f32, kind="ExternalInput").ap()
    out = nc.dram_tensor("out", [_BL, _OL], f32, kind="ExternalOutput").ap()

    with tile.TileContext(nc) as tc:
        with (
            tc.tile_pool(name="w8", bufs=_K) as w8p,
            tc.tile_pool(name="w16",
                         bufs=sum(-(-(_NIT - n) // 4) for n in _N8K)) as w16p,
            tc.tile_pool(name="x8", bufs=_NBT) as x8p,
            tc.tile_pool(name="x16", bufs=_NBT) as x16p,
            tc.tile_pool(name="singles", bufs=1) as singles,
            tc.tile_pool(name="acc", bufs=_NBT) as accp,
            tc.tile_pool(name="psum", bufs=8, space="PSUM") as psump,
        ):
            def load_w8(k, chunks=1):
                # chunked so the very first matmul waits on minimal bytes
                t_ = w8p.tile([128, _N8K[k], _OL], fp8, tag="w8",
                              name=f"w8_{k}")
                step = _N8K[k] // chunks
                for lo in range(0, _N8K[k], step):
                    nc.sync.dma_start(out=t_[:, lo:lo + step, :],
                                      in_=wT8[k][:, lo:lo + step, :])
                return t_

            def load_w16(k):
                # granules of <=4 blocks so a sweep can start on granule 0
                # while the rest still streams
                nb = _NIT - _N8K[k]
                gs = []
                for lo in range(0, nb, 4):
                    n = min(4, nb - lo)
                    t_ = w16p.tile([128, n, _OL], bf16, tag="w16",
                                   name=f"w16_{k}_{lo}")
                    nc.sync.dma_start(out=t_, in_=wT16[k][:, lo:lo + n, :])
                    gs.append(t_)
                return gs

            def load_attn(src, nm):
                t_ = singles.tile([128, _NBT, _K], f32, tag=nm, name=nm)
                nc.sync.dma_start(
                    out=t_,
                    in_=bass.AP(
                        tensor=src.tensor,
                        offset=src.offset,
                        ap=[[_K, 128], [128 * _K, _NBT], [1, _K]],
                    ),
                )
                return t_

            # ---- loads in need-order ----
            w8t = {0: load_w8(0, chunks=3)}
            x8ts = {}
            for t in range(_NBT):
                t_ = x8p.tile([128, _N8MAX, 128], fp8, tag="x8",
                              name=f"x8_{t}")
                if t == 0:
                    # first matmul waits only on the first pair's 32KB
                    nc.scalar.dma_start(out=t_[:, 0:2, :],
                                        in_=xT8[t][:, 0:2, :])
                    nc.scalar.dma_start(out=t_[:, 2:, :],
                                        in_=xT8[t][:, 2:, :])
                else:
                    nc.scalar.dma_start(out=t_, in_=xT8[t])
                x8ts[t] = t_
            attn_s_sb = load_attn(attn_s, "attn_s_sb")
            w8t[1] = load_w8(1)
            attn_sb = load_attn(attn, "attn_sb")
            w8t[2] = load_w8(2)
            w8t[3] = load_w8(3)
            x16ts = {}
            for t in range(_NBT):
                t_ = x16p.tile([128, _NX16, 128], bf16, tag="x16",
                               name=f"x16_{t}")
                nc.scalar.dma_start(out=t_, in_=xT16[t])
                x16ts[t] = t_
            # bf16 W in sweep-consumption order: k2, k3 (merged sweeps run
            # right after the fp8 phases), then k1, then k0 (epilogue)
            w16t = {}
            for k in (2, 3, 1, 0):
                w16t[k] = load_w16(k)

            bias_rep = singles.tile([128, _K, _OL], f32, tag="bias_rep",
                                    name="bias_rep")
            nc.gpsimd.dma_start(
                out=bias_rep,
                in_=bass.AP(
                    tensor=bias.tensor,
                    offset=bias.offset,
                    ap=[[0, 128], bias.ap[0], bias.ap[1]],
                ),
            )

            acc = {}

            def dr_passes(ps, k, t, start):
                for j in range(_N8K[k] // 2):
                    nc.tensor.matmul(
                        ps,
                        lhsT=x8ts[t][:, 2 * j:2 * j + 2, :],
                        rhs=w8t[k][:, 2 * j:2 * j + 2, :],
                        start=(start and j == 0), stop=False,
                        perf_mode=DR,
                    )

            def bf16_passes(ps, k, t, start):
                nb = _NIT - _N8K[k]
                xoff = _N8K[k] - _X16LO
                for ii in range(nb):
                    nc.tensor.matmul(
                        ps,
                        lhsT=x16ts[t][:, xoff + ii, :],
                        rhs=w16t[k][ii // 4][:, ii % 4, :],
                        start=(start and ii == 0), stop=(ii == nb - 1),
                    )

            # ---- fp8 prologue: expert 0 sweep, then expert 1 sweep.
            # NOTE: sweep-major order is deliberate. It keeps each b_tile's
            # DoubleRow burst short/DMA-paced; a t-major variant (6 DR
            # instrs back-to-back per tile) sustained enough double-pumped
            # MAC power across all 8 cores to trip a chip-wide clock clamp
            # (~20% slower whole-run, reproducibly). ----
            for t in range(_NBT):
                ps = psump.tile([128, _OL], f32, tag="ps", name=f"psA_{t}")
                np8 = _N8K[0] // 2
                for j in range(np8):
                    nc.tensor.matmul(
                        ps,
                        lhsT=x8ts[t][:, 2 * j:2 * j + 2, :],
                        rhs=w8t[0][:, 2 * j:2 * j + 2, :],
                        start=(j == 0), stop=(j == np8 - 1),
                        perf_mode=DR,
                    )
                at = accp.tile([128, _OL], f32, tag="acc", name=f"acc{t}")
                acc[t] = at
                nc.vector.tensor_scalar(
                    out=at, in0=ps, scalar1=attn_s_sb[:, t, 0:1],
                    scalar2=None, op0=MULT,
                )
            for t in range(_NBT):
                ps = psump.tile([128, _OL], f32, tag="ps", name=f"psA1_{t}")
                np8 = _N8K[1] // 2
                for j in range(np8):
                    nc.tensor.matmul(
                        ps,
                        lhsT=x8ts[t][:, 2 * j:2 * j + 2, :],
                        rhs=w8t[1][:, 2 * j:2 * j + 2, :],
                        start=(j == 0), stop=(j == np8 - 1),
                        perf_mode=DR,
                    )
                nc.vector.scalar_tensor_tensor(
                    out=acc[t], in0=ps, scalar=attn_s_sb[:, t, 1:2],
                    in1=acc[t], op0=MULT, op1=ADD,
                )

            # ---- merged sweeps: experts 2,3: fp8 + bf16 in one group ----
            for k in (2, 3):
                for t in range(_NBT):
                    ps = psump.tile([128, _OL], f32, tag="ps",
                                    name=f"ps_{k}_{t}")
                    dr_passes(ps, k, t, start=True)
                    bf16_passes(ps, k, t, start=False)
                    nc.vector.scalar_tensor_tensor(
                        out=acc[t], in0=ps, scalar=attn_s_sb[:, t, k:k + 1],
                        in1=acc[t], op0=MULT, op1=ADD,
                    )
                    nc.vector.scalar_tensor_tensor(
                        out=acc[t], in0=bias_rep[:, k, :],
                        scalar=attn_sb[:, t, k:k + 1], in1=acc[t],
                        op0=MULT, op1=ADD,
                    )

            # ---- bf16 halves of experts 1 then 0; bias; store at the end --
            for k in (1, 0):
                for t in range(_NBT):
                    ps = psump.tile([128, _OL], f32, tag="ps",
                                    name=f"psE_{k}_{t}")
                    # bias is independent of ps: issue first so the final
                    # tail is just one combine + the store
                    nc.vector.scalar_tensor_tensor(
                        out=acc[t], in0=bias_rep[:, k, :],
                        scalar=attn_sb[:, t, k:k + 1], in1=acc[t],
                        op0=MULT, op1=ADD,
                    )
                    bf16_passes(ps, k, t, start=True)
                    nc.vector.scalar_tensor_tensor(
                        out=acc[t], in0=ps, scalar=attn_s_sb[:, t, k:k + 1],
                        in1=acc[t], op0=MULT, op1=ADD,
                    )
                    if k == 0:
                        nc.sync.dma_start(
                            out=out[t * 128:(t + 1) * 128, :], in_=acc[t],
                        )

    nc.compile()
    return nc


def _get_program():
    if "nc" not in _CACHE:
        _CACHE["nc"] = _build_program()
    return _CACHE["nc"]


def _ensure_axon_hooks_importable():
    """bass_utils' trace branch imports antenv.axon_hooks, which the
    trimmed agent image may lack; stub it (hook=None) so a stray
    BASS_TRACE=1 degrades to an untraced run instead of crashing."""
    import sys
    import types

    try:
        import antenv.axon_hooks  # noqa: F401
        return
    except ImportError:
        pass
    mod = types.ModuleType("antenv.axon_hooks")
    mod._hook = None
    mod.get_axon_ntff_profile_hook = lambda: mod._hook

    def _set(h):
        mod._hook = h

    mod.set_axon_ntff_profile_hook = _set
    sys.modules["antenv.axon_hooks"] = mod
    try:
        import antenv
        antenv.axon_hooks = mod
    except ImportError:
        pass


def kernel(**inputs):
    global LAST_RESULTS
    import ml_dtypes
    from concourse.bass_utils import run_bass_kernel_spmd

    _ensure_axon_hooks_importable()

    x = np.ascontiguousarray(inputs["x"], dtype=np.float32)
    attn = np.ascontiguousarray(inputs["softmax_attention"], dtype=np.float32)
    w = np.ascontiguousarray(inputs["weight"], dtype=np.float32)
    b = np.ascontiguousarray(inputs["bias"], dtype=np.float32)

    nc = _get_program()
    in_maps = []
    for c in range(8):
        gb, go = divmod(c, _GRID_O)
        x_sl = x[gb * _BL:(gb + 1) * _BL]
        w_sl = w[:, go * _OL:(go + 1) * _OL, :]
        # tile-contiguous device layouts (see _build_program):
        # xT*[t, i_in, j, b_in] = x[t*128 + b_in, j*128 + i_in] (* scale)
        # wT*[i_in, j, o]       = W[k, o, j*128 + i_in] (* scale)
        xT8 = np.ascontiguousarray(
            (x_sl[:, :_N8MAX * 128] * _SX).T
            .reshape(_N8MAX, 128, _NBT, 128).transpose(2, 1, 0, 3)
        ).astype(ml_dtypes.float8_e4m3)
        xT16 = np.ascontiguousarray(
            x_sl[:, _X16LO * 128:].T
            .reshape(_NX16, 128, _NBT, 128).transpose(2, 1, 0, 3)
        ).astype(ml_dtypes.bfloat16)
        attn_c = np.ascontiguousarray(attn[gb * _BL:(gb + 1) * _BL])
        im = {
            "xT8": xT8,
            "xT16": xT16,
            "attn": attn_c,
            "attn_s": np.ascontiguousarray(attn_c / _STOT),
            "bias": np.ascontiguousarray(b[:, go * _OL:(go + 1) * _OL]),
        }
        for k in range(_K):
            cut = _N8K[k] * 128
            im[f"wT8_{k}"] = np.ascontiguousarray(
                (w_sl[k, :, :cut] * _SW).T
                .reshape(_N8K[k], 128, _OL).transpose(1, 0, 2)
            ).astype(ml_dtypes.float8_e4m3)
            im[f"wT16_{k}"] = np.ascontiguousarray(
                (w_sl[k, :, cut:] * _STOT).T
                .reshape(_NIT - _N8K[k], 128, _OL).transpose(1, 0, 2)
            ).astype(ml_dtypes.bfloat16)
        in_maps.append(im)

    res = run_bass_kernel_spmd(nc, in_maps, list(range(8)))
    LAST_RESULTS = res

    full = np.empty((_B, _OUT), dtype=np.float32)
    for c in range(8):
        gb, go = divmod(c, _GRID_O)
        full[gb * _BL:(gb + 1) * _BL, go * _OL:(go + 1) * _OL] = \
            res.results[c]["out"]
    return full


# revision 33
# speedup vs baseline: 1.2019x; 1.0402x over previous
"""DynamicLinear (MoE routing) Trainium2 Bass kernel.

Math (per sample b):
    out[b] = sum_k attn[b,k] * (x[b] @ W[k].T + bias[k])

Sharding: 8 cores in a 2x4 grid over (batch, out_features).
Each core computes out[b_half, o_quarter] from x[b_half] and
W[:, o_quarter, :] -- no cross-core communication.

Precision-hybrid contraction: per expert k, the first _N8K[k] (of 16)
128-wide contraction blocks run as fp8e4m3 DoubleRow matmuls (2
blocks per instruction, double-pumped PE = 2x bf16 rate), the rest
in bf16.  The total fp8 scale _SX*_SW = 2^17 is a power of two, so
the bf16 operands are pre-scaled by 2^17 exactly (lossless) and both
precisions accumulate into ONE PSUM bank per (expert, b_tile); the
inverse scale rides in the per-expert attn combine scalars
(attn_s = attn / 2^17).  _SW lifts the uniform weights out of
e4m3's subnormal range.  Full-batch rel_l2 vs fp32 ref: ~1.75e-2
(gate 2e-2); HW matches the CPU model to 7 digits.

Schedule: expert-outer sweeps.  k0 is split: its fp8 half runs first
(prologue needs only ~300KB before the first matmul -> early PE
start), its bf16 half runs last (epilogue).  k1..k3 run as merged
sweeps (DR pairs + bf16 blocks in one accumulation group).  One DVE
combine per group plus one DVE bias op per (k,t), paced well under
the PE; PSUM banks round-robin depth 8.
"""

import numpy as np

_B, _K, _IN, _OUT = 4096, 4, 2048, 2048
_GRID_B, _GRID_O = 2, 4
_BL = _B // _GRID_B      # 2048 batch rows per core
_OL = _OUT // _GRID_O    # 512 out cols per core
_NBT = _BL // 128        # 16 b tiles
_NIT = _IN // 128        # 16 contraction tiles
_N8K = (6, 6, 6, 4)      # fp8 blocks per expert (each even)
_N8MAX = max(_N8K)
_X16LO = min(_N8K)       # x16 tile holds blocks _X16LO.._NIT-1
_NX16 = _NIT - _X16LO
_STOT = 131072.0         # 2^17: total fp8 scale; bf16 w pre-scaled by it
_SX = 24.0
_SW = _STOT / _SX

_CACHE = {}
LAST_RESULTS = None


def _build_program():
    import concourse.bass as bass
    import concourse.tile as tile
    from concourse import bacc, mybir

    f32 = mybir.dt.float32
    bf16 = mybir.dt.bfloat16
    fp8 = mybir.dt.float8e4
    MULT = mybir.AluOpType.mult
    ADD = mybir.AluOpType.add
    DR = mybir.MatmulPerfMode.DoubleRow

    nc = bacc.Bacc("TRN2", target_bir_lowering=False, debug=False)
    xT8 = nc.dram_tensor("xT8", [_NBT, 128, _N8MAX, 128], fp8,
                         kind="ExternalInput").ap()
    xT16 = nc.dram_tensor("xT16", [_NBT, 128, _NX16, 128], bf16,
                          kind="ExternalInput").ap()
    attn = nc.dram_tensor("attn", [_BL, _K], f32, kind="ExternalInput").ap()
    attn_s = nc.dram_tensor("attn_s", [_BL, _K], f32,
                            kind="ExternalInput").ap()
    # per-expert fp8 W (ragged across k -> one tensor per k)
    wT8 = [nc.dram_tensor(f"wT8_{k}", [128, _N8K[k], _OL], fp8,
                          kind="ExternalInput").ap() for k in range(_K)]
    wT16 = [nc.dram_tensor(f"wT16_{k}", [128, _NIT - _N8K[k], _OL], bf16,
                           kind="ExternalInput").ap() for k in range(_K)]
    bias = nc.dram_tensor("bias", [_K, _OL], f32, kind="ExternalInput").ap()
    out = nc.dram_tensor("out", [_BL, _OL], f32, kind="ExternalOutput").ap()

    with tile.TileContext(nc) as tc:
        with (
            tc.tile_pool(name="w8", bufs=_K) as w8p,
            tc.tile_pool(name="w16",
                         bufs=sum(-(-(_NIT - n) // 4) for n in _N8K)) as w16p,
            tc.tile_pool(name="x8", bufs=_NBT) as x8p,
            tc.tile_pool(name="x16", bufs=_NBT) as x16p,
            tc.tile_pool(name="singles", bufs=1) as singles,
            tc.tile_pool(name="acc", bufs=_NBT) as accp,
            tc.tile_pool(name="psum", bufs=8, space="PSUM") as psump,
        ):
            def load_w8(k, chunks=1):
                # chunked so the very first matmul waits on minimal bytes
                t_ = w8p.tile([128, _N8K[k], _OL], fp8, tag="w8",
                              name=f"w8_{k}")
                step = _N8K[k] // chunks
                for lo in range(0, _N8K[k], step):
                    nc.sync.dma_start(out=t_[:, lo:lo + step, :],
                                      in_=wT8[k][:, lo:lo + step, :])
                return t_

            def load_w16(k):
                # granules of <=4 blocks so a sweep can start on granule 0
                # while the rest still streams
                nb = _NIT - _N8K[k]
                gs = []
                for lo in range(0, nb, 4):
                    n = min(4, nb - lo)
                    t_ = w16p.tile([128, n, _OL], bf16, tag="w16",
                                   name=f"w16_{k}_{lo}")
                    nc.sync.dma_start(out=t_, in_=wT16[k][:, lo:lo + n, :])
                    gs.append(t_)
                return gs

            def load_attn(src, nm):
                t_ = singles.tile([128, _NBT, _K], f32, tag=nm, name=nm)
                nc.sync.dma_start(
                    out=t_,
                    in_=bass.AP(
                        tensor=src.tensor,
                        offset=src.offset,
                        ap=[[_K, 128], [128 * _K, _NBT], [1, _K]],
                    ),
                )
                return t_

            # ---- loads in need-order ----
            w8t = {0: load_w8(0, chunks=3)}
            x8ts = {}
            for t in range(_NBT):
                t_ = x8p.tile([128, _N8MAX, 128], fp8, tag="x8",
                              name=f"x8_{t}")
                if t == 0:
                    # first matmul waits only on the first pair's 32KB
                    nc.scalar.dma_start(out=t_[:, 0:2, :],
                                        in_=xT8[t][:, 0:2, :])
                    nc.scalar.dma_start(out=t_[:, 2:, :],
                                        in_=xT8[t][:, 2:, :])
                else:
                    nc.scalar.dma_start(out=t_, in_=xT8[t])
                x8ts[t] = t_
            attn_s_sb = load_attn(attn_s, "attn_s_sb")
            w8t[1] = load_w8(1)
            attn_sb = load_attn(attn, "attn_sb")
            w8t[2] = load_w8(2)
            w8t[3] = load_w8(3)
            x16ts = {}
            for t in range(_NBT):
                t_ = x16p.tile([128, _NX16, 128], bf16, tag="x16",
                               name=f"x16_{t}")
                nc.scalar.dma_start(out=t_, in_=xT16[t])
                x16ts[t] = t_
            # bf16 W in sweep-consumption order: k2, k3 (merged sweeps run
            # right after the fp8 phases), then k1, then k0 (epilogue)
            w16t = {}
            for k in (2, 3, 1, 0):
                w16t[k] = load_w16(k)

            bias_rep = singles.tile([128, _K, _OL], f32, tag="bias_rep",
                                    name="bias_rep")
            nc.sync.dma_start(
                out=bias_rep,
                in_=bass.AP(
                    tensor=bias.tensor,
                    offset=bias.offset,
                    ap=[[0, 128], bias.ap[0], bias.ap[1]],
                ),
            )

            acc = {}

            def dr_passes(ps, k, t, start):
                for j in range(_N8K[k] // 2):
                    nc.tensor.matmul(
                        ps,
                        lhsT=x8ts[t][:, 2 * j:2 * j + 2, :],
                        rhs=w8t[k][:, 2 * j:2 * j + 2, :],
                        start=(start and j == 0), stop=False,
                        perf_mode=DR,
                    )

            def bf16_passes(ps, k, t, start):
                nb = _NIT - _N8K[k]
                xoff = _N8K[k] - _X16LO
                for ii in range(nb):
                    nc.tensor.matmul(
                        ps,
                        lhsT=x16ts[t][:, xoff + ii, :],
                        rhs=w16t[k][ii // 4][:, ii % 4, :],
                        start=(start and ii == 0), stop=(ii == nb - 1),
                    )

            # ---- fp8 prologue: expert 0 sweep, then expert 1 sweep.
            # NOTE: sweep-major order is deliberate. It keeps each b_tile's
            # DoubleRow burst short/DMA-paced; a t-major variant (6 DR
            # instrs back-to-back per tile) sustained enough double-pumped
            # MAC power across all 8 cores to trip a chip-wide clock clamp
            # (~20% slower whole-run, reproducibly). ----
            for t in range(_NBT):
                ps = psump.tile([128, _OL], f32, tag="ps", name=f"psA_{t}")
                np8 = _N8K[0] // 2
                for j in range(np8):
                    nc.tensor.matmul(
                        ps,
                        lhsT=x8ts[t][:, 2 * j:2 * j + 2, :],
                        rhs=w8t[0][:, 2 * j:2 * j + 2, :],
                        start=(j == 0), stop=(j == np8 - 1),
                        perf_mode=DR,
                    )
                at = accp.tile([128, _OL], f32, tag="acc", name=f"acc{t}")
                acc[t] = at
                nc.vector.tensor_scalar(
                    out=at, in0=ps, scalar1=attn_s_sb[:, t, 0:1],
                    scalar2=None, op0=MULT,
                )
            for t in range(_NBT):
                ps = psump.tile([128, _OL], f32, tag="ps", name=f"psA1_{t}")
                np8 = _N8K[1] // 2
                for j in range(np8):
                    nc.tensor.matmul(
                        ps,
                        lhsT=x8ts[t][:, 2 * j:2 * j + 2, :],
                        rhs=w8t[1][:, 2 * j:2 * j + 2, :],
                        start=(j == 0), stop=(j == np8 - 1),
                        perf_mode=DR,
                    )
                nc.vector.scalar_tensor_tensor(
                    out=acc[t], in0=ps, scalar=attn_s_sb[:, t, 1:2],
                    in1=acc[t], op0=MULT, op1=ADD,
                )

            # ---- merged sweeps: experts 2,3: fp8 + bf16 in one group ----
            for k in (2, 3):
                for t in range(_NBT):
                    ps = psump.tile([128, _OL], f32, tag="ps",
                                    name=f"ps_{k}_{t}")
                    dr_passes(ps, k, t, start=True)
                    bf16_passes(ps, k, t, start=False)
                    nc.vector.scalar_tensor_tensor(
                        out=acc[t], in0=ps, scalar=attn_s_sb[:, t, k:k + 1],
                        in1=acc[t], op0=MULT, op1=ADD,
                    )
                    nc.vector.scalar_tensor_tensor(
                        out=acc[t], in0=bias_rep[:, k, :],
                        scalar=attn_sb[:, t, k:k + 1], in1=acc[t],
                        op0=MULT, op1=ADD,
                    )

            # ---- bf16 halves of experts 1 then 0; bias; store at the end --
            for k in (1, 0):
                for t in range(_NBT):
                    ps = psump.tile([128, _OL], f32, tag="ps",
                                    name=f"psE_{k}_{t}")
                    # bias is independent of ps: issue first so the final
                    # tail is just one combine + the store
                    nc.vector.scalar_tensor_tensor(
                        out=acc[t], in0=bias_rep[:, k, :],
                        scalar=attn_sb[:, t, k:k + 1], in1=acc[t],
                        op0=MULT, op1=ADD,
                    )
                    bf16_passes(ps, k, t, start=True)
                    nc.vector.scalar_tensor_tensor(
                        out=acc[t], in0=ps, scalar=attn_s_sb[:, t, k:k + 1],
                        in1=acc[t], op0=MULT, op1=ADD,
                    )
                    if k == 0:
                        nc.sync.dma_start(
                            out=out[t * 128:(t + 1) * 128, :], in_=acc[t],
                        )

    nc.compile()
    return nc


def _get_program():
    if "nc" not in _CACHE:
        _CACHE["nc"] = _build_program()
    return _CACHE["nc"]


def _ensure_axon_hooks_importable():
    """bass_utils' trace branch imports antenv.axon_hooks, which the
    trimmed agent image may lack; stub it (hook=None) so a stray
    BASS_TRACE=1 degrades to an untraced run instead of crashing."""
    import sys
    import types

    try:
        import antenv.axon_hooks  # noqa: F401
        return
    except ImportError:
        pass
    mod = types.ModuleType("antenv.axon_hooks")
    mod._hook = None
    mod.get_axon_ntff_profile_hook = lambda: mod._hook

    def _set(h):
        mod._hook = h

    mod.set_axon_ntff_profile_hook = _set
    sys.modules["antenv.axon_hooks"] = mod
    try:
        import antenv
        antenv.axon_hooks = mod
    except ImportError:
        pass


def kernel(**inputs):
    global LAST_RESULTS
    import ml_dtypes
    from concourse.bass_utils import run_bass_kernel_spmd

    _ensure_axon_hooks_importable()

    x = np.ascontiguousarray(inputs["x"], dtype=np.float32)
    attn = np.ascontiguousarray(inputs["softmax_attention"], dtype=np.float32)
    w = np.ascontiguousarray(inputs["weight"], dtype=np.float32)
    b = np.ascontiguousarray(inputs["bias"], dtype=np.float32)

    nc = _get_program()
    in_maps = []
    for c in range(8):
        gb, go = divmod(c, _GRID_O)
        x_sl = x[gb * _BL:(gb + 1) * _BL]
        w_sl = w[:, go * _OL:(go + 1) * _OL, :]
        # tile-contiguous device layouts (see _build_program):
        # xT*[t, i_in, j, b_in] = x[t*128 + b_in, j*128 + i_in] (* scale)
        # wT*[i_in, j, o]       = W[k, o, j*128 + i_in] (* scale)
        xT8 = np.ascontiguousarray(
            (x_sl[:, :_N8MAX * 128] * _SX).T
            .reshape(_N8MAX, 128, _NBT, 128).transpose(2, 1, 0, 3)
        ).astype(ml_dtypes.float8_e4m3)
        xT16 = np.ascontiguousarray(
            x_sl[:, _X16LO * 128:].T
            .reshape(_NX16, 128, _NBT, 128).transpose(2, 1, 0, 3)
        ).astype(ml_dtypes.bfloat16)
        attn_c = np.ascontiguousarray(attn[gb * _BL:(gb + 1) * _BL])
        im = {
            "xT8": xT8,
            "xT16": xT16,
            "attn": attn_c,
            "attn_s": np.ascontiguousarray(attn_c / _STOT),
            "bias": np.ascontiguousarray(b[:, go * _OL:(go + 1) * _OL]),
        }
        for k in range(_K):
            cut = _N8K[k] * 128
            im[f"wT8_{k}"] = np.ascontiguousarray(
                (w_sl[k, :, :cut] * _SW).T
                .reshape(_N8K[k], 128, _OL).transpose(1, 0, 2)
            ).astype(ml_dtypes.float8_e4m3)
            im[f"wT16_{k}"] = np.ascontiguousarray(
                (w_sl[k, :, cut:] * _STOT).T
                .reshape(_NIT - _N8K[k], 128, _OL).transpose(1, 0, 2)
            ).astype(ml_dtypes.bfloat16)
        in_maps.append(im)

    res = run_bass_kernel_spmd(nc, in_maps, list(range(8)))
    LAST_RESULTS = res

    full = np.empty((_B, _OUT), dtype=np.float32)
    for c in range(8):
        gb, go = divmod(c, _GRID_O)
        full[gb * _BL:(gb + 1) * _BL, go * _OL:(go + 1) * _OL] = \
            res.results[c]["out"]
    return full
